# revision 41
# baseline (speedup 1.0000x reference)
"""DCNv3 (deformable conv v3) forward as a Bass/Tile kernel for Trainium2.

Contract: kernel(**inputs) takes the FULL inputs of reference.setup_inputs()
and returns the FULL (8, 64, 64, 128) output. The batch dim (8) is
data-parallel across 8 NeuronCores; each core runs an identical single-image
program (no collectives).

Algorithm (validated vs the jax reference in numpy, rel err ~4e-6):
  x_proj = x @ w_in + b_in
  x1     = gelu(LN(dwconv3x3(x) + dw_b) * ln_g + ln_b)
  offs   = x1 @ w_off + b_off        (per group g, point p: (ox, oy), |o|<1)
  e      = exp(x1 @ w_msk + b_msk);  m = e / sum_p e
  Bilinear sampling of point p at (h+1+ky+oy, w+1+kx+ox) decomposes into
  per-axis 3-tap tents  t[-1]=relu(-o), t[0]=1-|o|, t[1]=relu(o), so the
  mask-weighted sample sum collapses to a 5x5 shift window:
     out[pos, (g,c)] = sum_{sy,sx in [-2,2]} A[(g,sy,sx), pos] *
                       xproj_pad2[pos + (sy,sx), (g,c)]
  The 9 (dy,dx) tent-product terms are re-expressed in the 9-product basis
     {m, m*tymn, m*typ, m*txmn, m*txp, m*tymn*txmn, m*tymn*txp,
      m*typ*txmn, m*typ*txp},   tmn=min(o,0), tp=max(o,0),
  whose (constant) scatter matrices fold the basis-change coefficients, so
  the on-chip work is 4 one-scalar tensor_scalar tents (4x DVE mode) and 8
  elementwise products instead of 6 slow-path tent ops and 12 products.
  xproj is padded by 2 (inner ring = conv pad inside the sampling grid,
  outer ring = zeros = grid_sample zero padding), making all window reads
  in-bounds with no boundary special cases.
  final  = out @ w_out + b_out

Layout: channel-major [C on partitions, positions on the free axis], so all
channel contractions are natural matmuls. Matmul operands are bf16 (full PE
rate + FWL weight loads); all accumulation (PSUM) is fp32. x^T arrives via
the DMA xbar transpose (no PE/ACT cost). The output projection is fused
into the apply loop: final = sum_s (A_s (*) img_s) @ w_out accumulates
across the 25 shifts in PSUM; per-shift A-broadcast tiles are consumed
three ways to balance engines: evicted to SBUF by ACT, evicted by the
(otherwise idle) Pool engine, or multiplied straight out of PSUM by DVE.
A-build and apply phases share the schedule (two concurrently-open PSUM
pools) so their dependency bubbles overlap.
"""

from contextlib import ExitStack

import ml_dtypes
import numpy as np

import concourse.bass as bass
import concourse.mybir as mybir
import concourse.tile as tile
from concourse._compat import with_exitstack

N, H, W, C, G, K = 8, 64, 64, 128, 8, 3
GC = C // G            # 16
P = K * K              # 9
POS = H * W            # 4096
HP, WP = H + 2, W + 2            # dwconv pad-1 grid (66)
HP2, WP2 = H + 4, W + 4          # sampling pad-2 grid (68)
EPS = 1e-6
NS = 25                          # 5x5 shift window
NH1, NH2 = 13, 12                # A row split: s in [0,13), [13,25)
R1, R2 = G * NH1, G * NH2        # 104, 96 partition rows of the two A halves
NB = 9                           # tent-product basis size
F32 = mybir.dt.float32
BF16 = mybir.dt.bfloat16
FP8 = mybir.dt.float8e4
NPBF = ml_dtypes.bfloat16
NPF8 = ml_dtypes.float8_e4m3

# packed-constant column layouts (see _host_constants): one DMA per pack
# instead of ~29 serial HWDGE loads, which gated startup by ~12us.
# pkE (bf16): dwdiag | w_in | invc_col | ones_row | b_out_row
_PKE_DW, _PKE_WIN, _PKE_INVC, _PKE_ONES, _PKE_BOUT, _PKE_BINR, _PKE_N = (
    0, P * C, P * C + C, P * C + C + 1, P * C + 2 * C + 1, P * C + 3 * C + 1,
    P * C + 4 * C + 1)
# pkL (bf16): w_offx | w_offy | w_msk | w_out
_PKL_OFFX, _PKL_OFFY, _PKL_MSK, _PKL_WOUT, _PKL_N = (
    0, G * P, 2 * G * P, 3 * G * P, 3 * G * P + C)
# pk8 (fp8e4, exact 0/±small-int constants): m1 | m2 | ea1 | ea2 | egg
_PK8_M1, _PK8_M2, _PK8_EA1, _PK8_EA2, _PK8_EGG, _PK8_N = (
    0, NB * R1, NB * (R1 + R2), NB * (R1 + R2) + NH1 * C,
    NB * (R1 + R2) + (NH1 + NH2) * C, NB * (R1 + R2) + (NH1 + NH2) * C + G * P)
# pkV (fp32 columns): b_in | dw_b | ln_g | ln_b | b_offx | b_offy | b_msk
_PKV_N = 7

CHUNK = 512                      # free-dim chunk for the build phase
NCH = POS // CHUNK               # 8

AF = mybir.ActivationFunctionType
OP = mybir.AluOpType

# Per-shift handling of the A-broadcast PSUM tile in the apply loop (Pool
# cannot touch PSUM, so every evict is on ACT):
#   'A' = ACT evicts to bf16 SBUF, DVE multiplies at 2x
#   'M' = ACT evicts to bf16 SBUF, Pool multiplies (slow but otherwise idle)
#   'D' = DVE multiplies straight out of PSUM (1x, but no evict at all)
APPLY_MODE = ['D', 'D', 'M', 'D', 'A',
              'D', 'A', 'M', 'D', 'A',
              'D', 'M', 'A', 'M', 'D',
              'M', 'D', 'A', 'D', 'A',
              'D', 'M', 'A', 'M', 'D']

# basis-change coefficients: d-term (dy_idx, dx_idx) -> {basis index: coeff}
# with stored tents tmn=min(o,0), tp=max(o,0) and actual taps
# t[-1]=-tmn, t[0]=1+tmn-tp, t[1]=tp.
_COEFF = {
    (0, 0): {5: 1.0},
    (0, 1): {1: -1.0, 5: -1.0, 6: 1.0},
    (0, 2): {6: -1.0},
    (1, 0): {3: -1.0, 5: -1.0, 7: 1.0},
    (1, 1): {0: 1.0, 1: 1.0, 2: -1.0, 3: 1.0, 4: -1.0,
             5: 1.0, 6: -1.0, 7: -1.0, 8: 1.0},
    (1, 2): {4: 1.0, 6: 1.0, 8: -1.0},
    (2, 0): {7: -1.0},
    (2, 1): {2: 1.0, 7: 1.0, 8: -1.0},
    (2, 2): {8: 1.0},
}


# --------------------------------------------------------------------------
# host-side constant matrices
# --------------------------------------------------------------------------

def _host_constants(inputs):
    dw_w = np.asarray(inputs["dw_w"], np.float32)        # (3,3,1,C) [ky,kx]
    w_off = np.asarray(inputs["w_off"], np.float32)      # (C, G*P*2)
    b_off = np.asarray(inputs["b_off"], np.float32)      # (G*P*2,)

    # depthwise weights as 9 diagonal matrices, c-major: [c_row, s, c_col]
    dwdiag = np.zeros((C, P, C), np.float32)
    for s in range(P):
        ky, kx = s // 3, s % 3
        dwdiag[np.arange(C), s, np.arange(C)] = dw_w[ky, kx, 0]

    w_offx = np.ascontiguousarray(w_off[:, 0::2])        # (C, 72)
    w_offy = np.ascontiguousarray(w_off[:, 1::2])
    b_offx = np.ascontiguousarray(b_off[0::2])           # (72,)
    b_offy = np.ascontiguousarray(b_off[1::2])

    # block-ones matrix: one matmul produces the group sums broadcast to
    # all 72 (g,p) rows at once (softmax denominator)
    egg = np.zeros((G * P, G * P), np.float32)
    for g in range(G):
        egg[g * P:(g + 1) * P, g * P:(g + 1) * P] = 1.0

    # A-scatter matrices over the 9-product basis: basis term b of point
    # (g,p) lands in A row (g, s), s = (p%3 + dy)*5 + (p//3 + dx) (x-major
    # p!), weighted by the basis-change coefficient of d-term (dy,dx).
    m1 = np.zeros((G * P, NB, R1), np.float32)
    m2 = np.zeros((G * P, NB, R2), np.float32)
    for (dy, dx), cs in _COEFF.items():
        for g in range(G):
            for p in range(P):
                s = (p % 3 + dy) * 5 + (p // 3 + dx)
                for b, coef in cs.items():
                    if s < NH1:
                        m1[g * P + p, b, g * NH1 + s] += coef
                    else:
                        m2[g * P + p, b, g * NH2 + (s - NH1)] += coef

    # A-broadcast matrices: A row (g, s) -> output row (g*GC + c)
    ea1 = np.zeros((R1, NH1, C), np.float32)
    ea2 = np.zeros((R2, NH2, C), np.float32)
    for g in range(G):
        for sl in range(NH1):
            ea1[g * NH1 + sl, sl, g * GC:(g + 1) * GC] = 1.0
        for sl in range(NH2):
            ea2[g * NH2 + sl, sl, g * GC:(g + 1) * GC] = 1.0

    # ---- pack into 4 DMA-able constants ------------------------------
    pkE = np.zeros((C, _PKE_N), np.float32)
    pkE[:, _PKE_DW:_PKE_DW + P * C] = dwdiag.reshape(C, P * C)
    pkE[:, _PKE_WIN:_PKE_WIN + C] = np.asarray(inputs["w_in"], np.float32)
    pkE[:, _PKE_INVC] = 1.0 / C
    pkE[0, _PKE_ONES:_PKE_ONES + C] = 1.0
    pkE[0, _PKE_BOUT:_PKE_BOUT + C] = np.asarray(inputs["b_out"], np.float32)
    pkE[0, _PKE_BINR:_PKE_BINR + C] = np.asarray(inputs["b_in"], np.float32)

    pkL = np.zeros((C, _PKL_N), np.float32)
    pkL[:, _PKL_OFFX:_PKL_OFFX + G * P] = w_offx
    pkL[:, _PKL_OFFY:_PKL_OFFY + G * P] = w_offy
    pkL[:, _PKL_MSK:_PKL_MSK + G * P] = np.asarray(inputs["w_msk"], np.float32)
    pkL[:, _PKL_WOUT:_PKL_WOUT + C] = np.asarray(inputs["w_out"], np.float32)

    pk8 = np.zeros((C, _PK8_N), np.float32)
    pk8[:G * P, _PK8_M1:_PK8_M1 + NB * R1] = m1.reshape(G * P, NB * R1)
    pk8[:G * P, _PK8_M2:_PK8_M2 + NB * R2] = m2.reshape(G * P, NB * R2)
    pk8[:R1, _PK8_EA1:_PK8_EA1 + NH1 * C] = ea1.reshape(R1, NH1 * C)
    pk8[:R2, _PK8_EA2:_PK8_EA2 + NH2 * C] = ea2.reshape(R2, NH2 * C)
    pk8[:G * P, _PK8_EGG:_PK8_EGG + G * P] = egg

    pkV = np.zeros((C, _PKV_N), np.float32)
    pkV[:, 0] = np.asarray(inputs["b_in"], np.float32)
    pkV[:, 1] = np.asarray(inputs["dw_b"], np.float32)
    pkV[:, 2] = np.asarray(inputs["ln_g"], np.float32)
    pkV[:, 3] = np.asarray(inputs["ln_b"], np.float32)
    pkV[:G * P, 4] = b_off[0::2]
    pkV[:G * P, 5] = b_off[1::2]
    pkV[:G * P, 6] = np.asarray(inputs["b_msk"], np.float32)

    return {"pkE": pkE.astype(NPBF), "pkL": pkL.astype(NPBF),
            "pk8": pk8.astype(NPF8), "pkV": pkV}


# --------------------------------------------------------------------------
# the per-core Tile program
# --------------------------------------------------------------------------

@with_exitstack
def _dcn_tile(ctx: ExitStack, tc: tile.TileContext, io: dict):
    nc = tc.nc
    ctx.enter_context(nc.allow_low_precision(
        reason="bf16 matmul operands; accumulation stays fp32 in PSUM and "
               "in the fp32 sampling accumulator"))

    persist = ctx.enter_context(tc.tile_pool(name="persist", bufs=1))
    temps = ctx.enter_context(tc.tile_pool(name="temps", bufs=3))
    tents = ctx.enter_context(tc.tile_pool(name="tents", bufs=2))

    # ---- packed constant loads ----------------------------------------
    # x^T transpose rides the DMA xbar; two separate landing tiles so each
    # restride copy depends on only its own transpose (transposed writes
    # are tracked whole-tile)
    xt_half = [persist.tile([C, POS // 2], BF16, tag=f"xt_half{h}",
                            name=f"xt_half{h}") for h in range(2)]

    def transpose_half(hf, eng):
        eng.dma_start(
            out=xt_half[hf],
            in_=bass.AP(tensor=io["x"].tensor,
                        offset=io["x"].offset + hf * (POS // 2) * C,
                        ap=[[C, POS // 2], [1, C]]),
            transpose=True)

    pkE_t = persist.tile([C, _PKE_N], BF16, tag="pkE")
    pkL_t = persist.tile([C, _PKL_N], BF16, tag="pkL")
    pk8_t = persist.tile([C, _PK8_N], FP8, tag="pk8")
    pkV_t = persist.tile([C, _PKV_N], F32, tag="pkV")

    # spread the loads over all DMA-issuing queues (ACT/SP HWDGE + Pool
    # SWDGE) so the tile scheduler can't chain them behind each other
    transpose_half(0, nc.sync)
    nc.sync.dma_start(out=pkE_t, in_=io["pkE"])
    transpose_half(1, nc.scalar)
    nc.scalar.dma_start(out=pk8_t, in_=io["pk8"])
    nc.gpsimd.dma_start(out=pkV_t, in_=io["pkV"])
    nc.gpsimd.dma_start(out=pkL_t, in_=io["pkL"])

    # views into the packs (same shapes the rest of the program expects)
    dwdiag = pkE_t[:, _PKE_DW:_PKE_DW + P * C].rearrange(
        "c (s k) -> c s k", k=C)                 # [c_row, s, c_col]
    w_in = pkE_t[:, _PKE_WIN:_PKE_WIN + C]
    invc_col = pkE_t[:, _PKE_INVC:_PKE_INVC + 1]
    ones_row = pkE_t[0:1, _PKE_ONES:_PKE_ONES + C]
    b_out_row = pkE_t[0:1, _PKE_BOUT:_PKE_BOUT + C]
    b_in_row = pkE_t[0:1, _PKE_BINR:_PKE_BINR + C]
    b_in = pkV_t[:, 0:1]
    dw_b = pkV_t[:, 1:2]
    ln_g = pkV_t[:, 2:3]
    ln_b = pkV_t[:, 3:4]
    b_offx = pkV_t[0:G * P, 4:5]
    b_offy = pkV_t[0:G * P, 5:6]
    b_msk = pkV_t[0:G * P, 6:7]
    w_offx = pkL_t[:, _PKL_OFFX:_PKL_OFFX + G * P]
    w_offy = pkL_t[:, _PKL_OFFY:_PKL_OFFY + G * P]
    w_msk = pkL_t[:, _PKL_MSK:_PKL_MSK + G * P]
    w_out = pkL_t[:, _PKL_WOUT:_PKL_WOUT + C]
    m1 = pk8_t[0:G * P, _PK8_M1:_PK8_M1 + NB * R1].rearrange(
        "p (b r) -> p b r", r=R1)
    m2 = pk8_t[0:G * P, _PK8_M2:_PK8_M2 + NB * R2].rearrange(
        "p (b r) -> p b r", r=R2)
    ea1 = pk8_t[0:R1, _PK8_EA1:_PK8_EA1 + NH1 * C].rearrange(
        "r (s c) -> r s c", c=C)
    ea2 = pk8_t[0:R2, _PK8_EA2:_PK8_EA2 + NH2 * C].rearrange(
        "r (s c) -> r s c", c=C)
    egg = pk8_t[0:G * P, _PK8_EGG:_PK8_EGG + G * P]

    eps1 = persist.tile([1, 1], F32)
    nc.vector.memset(eps1, EPS)

    # ---- persistent activations ---------------------------------------
    xt_pad = persist.tile([C, HP, WP], BF16)     # x^T, conv-padded (66x66)
    xpj_pad = persist.tile([C, HP2, WP2], BF16)  # x_proj^T, pad-2 (68x68)
    a1 = [persist.tile([R1, CHUNK], BF16, tag=f"a1_{i}", name=f"a1_{i}")
          for i in range(NCH)]
    a2 = [persist.tile([R2, CHUNK], BF16, tag=f"a2_{i}", name=f"a2_{i}")
          for i in range(NCH)]

    # only the pad rings need zeroing; the interiors are fully overwritten
    nc.vector.memset(xt_pad[:, 0:1, :], 0.0)
    nc.vector.memset(xt_pad[:, HP - 1:HP, :], 0.0)
    nc.vector.memset(xt_pad[:, 1:HP - 1, 0:1], 0.0)
    nc.vector.memset(xt_pad[:, 1:HP - 1, WP - 1:WP], 0.0)
    nc.vector.memset(xpj_pad[:, 0:2, :], 0.0)
    nc.vector.memset(xpj_pad[:, HP2 - 2:HP2, :], 0.0)
    nc.vector.memset(xpj_pad[:, 2:HP2 - 2, 0:2], 0.0)
    nc.vector.memset(xpj_pad[:, 2:HP2 - 2, WP2 - 2:WP2], 0.0)

    # ---- stage 1: restride x^T into the padded grid (DVE 4x copies) ---
    nc.vector.tensor_copy(out=xt_pad[:, 1:1 + H // 2, 1:1 + W],
                          in_=xt_half[0])
    nc.vector.tensor_copy(out=xt_pad[:, 1 + H // 2:1 + H, 1:1 + W],
                          in_=xt_half[1])

    # ---- stage 2: x_proj into the pad-2 grid --------------------------
    with tc.tile_pool(name="ps_s12", bufs=2, space="PSUM") as psum:
        for ch in range(NCH):        # chunk = 8 h-rows
            h0 = ch * 8
            rhs = xt_pad[:, 1 + h0:1 + h0 + 8, 1:1 + W]
            ps = psum.tile([C, CHUNK], F32, tag="ps_proj")
            nc.tensor.matmul(ps, w_in, rhs, start=True, stop=True)
            nc.scalar.activation(
                out=xpj_pad[:, 2 + h0:2 + h0 + 8, 2:2 + W],
                in_=ps.rearrange("c (a b) -> c a b", b=W),
                func=AF.Identity, bias=b_in, scale=1.0)

    # ---- stages 3+4, interleaved ---------------------------------------
    # Two PSUM pools, both open across the whole fused phase so the
    # scheduler can interleave A-building and applying freely.
    with tc.tile_pool(name="ps3", bufs=1, space="PSUM") as psum, \
            tc.tile_pool(name="ps4", bufs=1, space="PSUM") as psum4:

        def build_a(ch):
            """dwconv + the [1,512] LayerNorm stats sub-chain.  Emitted
            BEFORE apply(ch-1) so the long serial LN chain percolates
            through ACT/DVE/Pool while PE grinds the apply matmuls."""
            h0 = ch * 8
            # depthwise conv via 9 diagonal matmuls
            psA = psum.tile([C, CHUNK], F32, tag="psdw", bufs=1, name="psA")
            for s in range(P):
                ky, kx = s // 3, s % 3
                rhs = xt_pad[:, h0 + ky:h0 + ky + 8, kx:kx + W]
                nc.tensor.matmul(psA, dwdiag[:, s, :], rhs,
                                 start=(s == 0), stop=(s == P - 1))
            x1c = temps.tile([C, CHUNK], BF16, tag="x1c")
            nc.scalar.activation(out=x1c, in_=psA, func=AF.Identity,
                                 bias=dw_b, scale=1.0)

            # LayerNorm stats over channels (partition dim) via 1/C cols;
            # x1sq on the Pool engine so it doesn't queue behind the
            # previous chunk's apply muls on DVE.
            x1sq = temps.tile([C, CHUNK], BF16, tag="scr")
            nc.gpsimd.tensor_mul(out=x1sq, in0=x1c, in1=x1c)
            pstat = psum.tile([33, CHUNK], F32, tag="psdw", bufs=1,
                              name="pstat")
            nc.tensor.matmul(pstat[0:1, :], invc_col, x1c, start=True,
                             stop=True)
            mean_r = temps.tile([1, CHUNK], BF16, tag="mean_r")
            nc.scalar.copy(out=mean_r, in_=pstat[0:1, :])
            nc.tensor.matmul(pstat[32:33, :], invc_col, x1sq, start=True,
                             stop=True)
            tmp_r = temps.tile([1, CHUNK], F32, tag="tmp_r")
            nc.scalar.activation(out=tmp_r, in_=pstat[0:1, :],
                                 func=AF.Square)
            nc.vector.tensor_sub(out=tmp_r, in0=pstat[32:33, :], in1=tmp_r)
            nc.scalar.activation(out=tmp_r, in_=tmp_r, func=AF.Sqrt,
                                 bias=eps1, scale=1.0)            # std
            rstd_r = temps.tile([1, CHUNK], BF16, tag="rstd_r")
            nc.vector.reciprocal(out=rstd_r, in_=tmp_r)           # rstd
            mrs_r = temps.tile([1, CHUNK], BF16, tag="mrs_r")
            nc.vector.tensor_mul(out=mrs_r, in0=mean_r, in1=rstd_r)
            return x1c, rstd_r, mrs_r

        def build_b(ch, x1c, rstd_r, mrs_r):
            h0 = ch * 8
            psR = psum.tile([C, CHUNK], F32, tag="psoff", bufs=2, name="psR")
            nc.tensor.matmul(psR, ones_row, rstd_r, start=True, stop=True)
            zc = temps.tile([C, CHUNK], F32, tag="scr2")
            nc.vector.tensor_mul(out=zc, in0=x1c, in1=psR)
            psM = psum.tile([C, CHUNK], F32, tag="psoff", bufs=2, name="psM")
            nc.tensor.matmul(psM, ones_row, mrs_r, start=True, stop=True)
            nc.vector.tensor_sub(out=zc, in0=zc, in1=psM)
            x1gc = temps.tile([C, CHUNK], BF16, tag="x1gc")
            nc.scalar.activation(out=x1gc, in_=zc, func=AF.Gelu,
                                 bias=ln_b, scale=ln_g)

            # offset projections; tents as one-scalar tensor_scalar ops on
            # bf16 SBUF evictions (4x DVE mode)
            psX = psum.tile([G * P, CHUNK], F32, tag="psoff", bufs=2,
                            name="psX")
            nc.tensor.matmul(psX, w_offx, x1gc, start=True, stop=True)
            oxs = tents.tile([G * P, CHUNK], BF16, tag="oxs")
            nc.scalar.activation(out=oxs, in_=psX, func=AF.Identity,
                                 bias=b_offx, scale=1.0)
            txmn = tents.tile([G * P, CHUNK], BF16, tag="txmn")
            txp = tents.tile([G * P, CHUNK], BF16, tag="txp")
            nc.vector.tensor_scalar(out=txmn, in0=oxs, scalar1=0.0,
                                    scalar2=None, op0=OP.min)
            nc.vector.tensor_scalar(out=txp, in0=oxs, scalar1=0.0,
                                    scalar2=None, op0=OP.max)
            psY = psum.tile([G * P, CHUNK], F32, tag="psoff", bufs=2,
                            name="psY")
            nc.tensor.matmul(psY, w_offy, x1gc, start=True, stop=True)
            oys = tents.tile([G * P, CHUNK], BF16, tag="oys")
            nc.scalar.activation(out=oys, in_=psY, func=AF.Identity,
                                 bias=b_offy, scale=1.0)
            tymn = tents.tile([G * P, CHUNK], BF16, tag="tymn")
            typ = tents.tile([G * P, CHUNK], BF16, tag="typ")
            nc.vector.tensor_scalar(out=tymn, in0=oys, scalar1=0.0,
                                    scalar2=None, op0=OP.min)
            nc.vector.tensor_scalar(out=typ, in0=oys, scalar1=0.0,
                                    scalar2=None, op0=OP.max)

            # normalized mask: e_n = exp(l + b) / group sum
            psE = psum.tile([G * P, CHUNK], F32, tag="psoff", bufs=2,
                            name="psE")
            nc.tensor.matmul(psE, w_msk, x1gc, start=True, stop=True)
            ec = temps.tile([G * P, CHUNK], BF16, tag="ec")
            nc.scalar.activation(out=ec, in_=psE, func=AF.Exp,
                                 bias=b_msk, scale=1.0)
            psB = psum.tile([G * P, CHUNK], F32, tag="psoff", bufs=2,
                            name="psB")
            nc.tensor.matmul(psB, egg, ec, start=True, stop=True)
            rec_b = temps.tile([G * P, CHUNK], BF16, tag="rec_b")
            nc.vector.reciprocal(out=rec_b, in_=psB)
            nc.vector.tensor_mul(out=ec, in0=ec, in1=rec_b)

            # 8 basis products (all-bf16 SBUF tensor_tensor, 2x mode)
            b1 = tents.tile([G * P, CHUNK], BF16, tag="b1")
            b2 = tents.tile([G * P, CHUNK], BF16, tag="b2")
            b3 = tents.tile([G * P, CHUNK], BF16, tag="b3")
            b4 = tents.tile([G * P, CHUNK], BF16, tag="b4")
            b5 = tents.tile([G * P, CHUNK], BF16, tag="b5")
            b6 = tents.tile([G * P, CHUNK], BF16, tag="b6")
            b7 = tents.tile([G * P, CHUNK], BF16, tag="b7")
            b8 = tents.tile([G * P, CHUNK], BF16, tag="b8")
            nc.vector.tensor_mul(out=b1, in0=ec, in1=tymn)
            nc.vector.tensor_mul(out=b2, in0=ec, in1=typ)
            nc.vector.tensor_mul(out=b3, in0=ec, in1=txmn)
            nc.vector.tensor_mul(out=b4, in0=ec, in1=txp)
            nc.gpsimd.tensor_mul(out=b5, in0=b1, in1=txmn)
            nc.vector.tensor_mul(out=b6, in0=b1, in1=txp)
            nc.gpsimd.tensor_mul(out=b7, in0=b2, in1=txmn)
            nc.vector.tensor_mul(out=b8, in0=b2, in1=txp)
            basis = [ec, b1, b2, b3, b4, b5, b6, b7, b8]

            psA1 = psum.tile([R1, CHUNK], F32, tag="psa", bufs=1,
                             name="psA1")
            for b in range(NB):
                nc.tensor.matmul(psA1, m1[:, b, :], basis[b],
                                 start=(b == 0), stop=(b == NB - 1))
            nc.scalar.copy(out=a1[ch], in_=psA1)
            psA2 = psum.tile([R2, CHUNK], F32, tag="psa", bufs=1,
                             name="psA2")
            for b in range(NB):
                nc.tensor.matmul(psA2, m2[:, b, :], basis[b],
                                 start=(b == 0), stop=(b == NB - 1))
            nc.scalar.copy(out=a2[ch], in_=psA2)

        def apply_chunk(ch):
            h0 = ch * 8            # 8 h-rows per 512-pos chunk
            # pos-major output accumulator: outp[p, q, c] is position
            # ch*512 + q*128 + p.  Initialized with the bias via rank-1
            # matmuls; each shift's product tile t becomes the lhsT of four
            # 128-col projection matmuls, so no output transpose is needed.
            outp = psum4.tile([C, 4, C], F32, tag="psoacc", bufs=1,
                              name="outp")
            for q in range(4):
                nc.tensor.matmul(outp[:, q, :], ones_row, b_out_row,
                                 start=True, stop=False)
            def product(s, mode):
                sy, sx = s // 5 - 2, s % 5 - 2
                if s < NH1:
                    lhsT, arows = ea1[:, s, :], a1[ch]
                else:
                    lhsT, arows = ea2[:, s - NH1, :], a2[ch]
                psBc = psum4.tile([C, CHUNK], F32, tag="psab", bufs=3,
                                  name="psBc")
                nc.tensor.matmul(psBc, lhsT, arows, start=True, stop=True)
                row = 2 + sy + h0
                img = xpj_pad[:, row:row + 8, 2 + sx:2 + sx + W]
                if mode == 'D':      # multiply straight out of PSUM
                    t = temps.tile([C, CHUNK], BF16, tag="t_app")
                    nc.vector.tensor_mul(out=t, in0=psBc, in1=img)
                elif mode == 'A':
                    ab = temps.tile([C, CHUNK], BF16, tag="ab_a")
                    nc.scalar.copy(out=ab, in_=psBc)
                    t = temps.tile([C, CHUNK], BF16, tag="t_app")
                    nc.vector.tensor_mul(out=t, in0=ab, in1=img)
                else:                # Pool multiplies; wout deferred
                    ab = temps.tile([C, CHUNK], BF16, tag="ab_p", bufs=6)
                    nc.scalar.copy(out=ab, in_=psBc)
                    t = temps.tile([C, CHUNK], BF16, tag="t_appm", bufs=6)
                    nc.gpsimd.tensor_mul(out=t, in0=ab, in1=img)
                return t

            def wout(t, last):
                for q in range(4):
                    nc.tensor.matmul(outp[:, q, :], t[:, q * C:(q + 1) * C],
                                     w_out, start=False, stop=last)

            # The slow Pool products would stall the in-order PSUM
            # accumulation, so their ea/evict/mul run interleaved with the
            # main pass but their wout groups accumulate at the very end.
            # final chunk: no later build overlaps the slow Pool muls, so
            # run its evicted shifts on ACT+DVE instead
            mode_l = APPLY_MODE if ch < NCH - 1 else [
                'A' if m == 'M' else m for m in APPLY_MODE]
            m_shifts = [s for s in range(NS) if mode_l[s] == 'M']
            o_shifts = [s for s in range(NS) if mode_l[s] != 'M']
            mq = list(m_shifts)
            m_prods = []
            for i, s in enumerate(o_shifts):
                if i % 2 == 0 and mq:
                    sm = mq.pop(0)
                    m_prods.append(product(sm, 'M'))
                wout(product(s, mode_l[s]),
                     last=(not m_prods and not mq and s == o_shifts[-1]))
            while mq:
                m_prods.append(product(mq.pop(0), 'M'))
            for j, t in enumerate(m_prods):
                wout(t, last=(j == len(m_prods) - 1))

            osb = temps.tile([C, 4, C], F32, tag="osb")
            if ch % 2 == 0:
                nc.vector.tensor_copy(out=osb, in_=outp)
            else:
                nc.scalar.copy(out=osb, in_=outp)
            pos0 = ch * CHUNK
            nc.sync.dma_start(
                out=bass.AP(tensor=io["out"].tensor,
                            offset=io["out"].offset + pos0 * C,
                            ap=[[C, C], [C * C, 4], [1, C]]),
                in_=osb)

        fronts = {0: build_a(0)}
        for ch in range(NCH):
            if ch + 1 < NCH:
                fronts[ch + 1] = build_a(ch + 1)
            if ch > 0:
                apply_chunk(ch - 1)
            build_b(ch, *fronts.pop(ch))
        apply_chunk(NCH - 1)


# --------------------------------------------------------------------------
# bass module build + public entry point
# --------------------------------------------------------------------------

# Hardware TPB instructions carry exactly ONE sync-wait slot (the
# NEURON_ISA_TPB_EVENTS struct).  Tile can emit several waits on one BIR
# instruction; walrus splits matmult waits across the LDWEIGHTS/MATMULT
# pair, but single-struct ops (Activation, ...) fail codegen with "Too many
# sync wait commands".  Move surplus waits onto standalone same-engine
# EventSemaphore instructions inserted immediately before the offender.
def _wait_cap(ins):
    t = type(ins).__name__
    if t == "InstEventSemaphore":
        return None
    return 1


def _split_surplus_waits(nc):
    import bass_rust
    n = 0
    for bb in nc.m.functions[0].blocks:
        out = []
        for ins in bb.instructions:
            si = getattr(ins, "sync_info", None)
            cap = _wait_cap(ins)
            if si is not None and cap is not None and len(si.on_wait) > cap:
                waits = list(si.on_wait)
                for i, w in enumerate(waits[:-cap]):
                    ev = mybir.InstEventSemaphore(
                        name=f"{ins.name}_xw{i}", ins=[], outs=[])
                    ev.engine = ins.engine
                    ev.sync_info = bass_rust.SyncInfo(on_wait=[w],
                                                     on_update=[])
                    nc.register_instruction(ev)
                    out.append(ev)
                    n += 1
                ins.sync_info = bass_rust.SyncInfo(
                    on_wait=waits[-cap:], on_update=list(si.on_update))
            out.append(ins)
        bb.instructions = out
    return n


_CACHED = {}


def _build_bass():
    if "nc" in _CACHED:
        return _CACHED["nc"]
    nc = bass.Bass()
    io = {}
    specs = {
        "x": ((POS, C), BF16),
        "pkE": ((C, _PKE_N), BF16),
        "pkL": ((C, _PKL_N), BF16),
        "pk8": ((C, _PK8_N), FP8),
        "pkV": ((C, _PKV_N), F32),
    }
    for name, (shape, dt) in specs.items():
        io[name] = nc.dram_tensor(name, list(shape), dt,
                                  kind="ExternalInput").ap()
    io["out"] = nc.dram_tensor("out", [POS, C], F32,
                               kind="ExternalOutput").ap()
    with tile.TileContext(nc) as tc:
        _dcn_tile(tc, io)
    _split_surplus_waits(nc)
    _CACHED["nc"] = nc
    return nc


def make_in_maps(inputs):
    consts = _host_constants(inputs)
    x = np.asarray(inputs["x"], np.float32).reshape(N, POS, C).astype(NPBF)
    return [{**consts, "x": np.ascontiguousarray(x[i])} for i in range(N)]


def kernel(**inputs):
    nc = _build_bass()
    in_maps = make_in_maps(inputs)
    from concourse.bass_utils import run_bass_kernel_spmd
    res = run_bass_kernel_spmd(nc, in_maps, list(range(N)))
    out = np.stack([res.results[i]["out"] for i in range(N)])
    return out.reshape(N, H, W, C).astype(np.float32)



# revision 42
# speedup vs baseline: 1.0183x; 1.0183x over previous
"""DCNv3 (deformable conv v3) forward as a Bass/Tile kernel for Trainium2.

Contract: kernel(**inputs) takes the FULL inputs of reference.setup_inputs()
and returns the FULL (8, 64, 64, 128) output. The batch dim (8) is
data-parallel across 8 NeuronCores; each core runs an identical single-image
program (no collectives).

Algorithm (validated vs the jax reference in numpy, rel err ~4e-6):
  x_proj = x @ w_in + b_in
  x1     = gelu(LN(dwconv3x3(x) + dw_b) * ln_g + ln_b)
  offs   = x1 @ w_off + b_off        (per group g, point p: (ox, oy), |o|<1)
  e      = exp(x1 @ w_msk + b_msk);  m = e / sum_p e
  Bilinear sampling of point p at (h+1+ky+oy, w+1+kx+ox) decomposes into
  per-axis 3-tap tents  t[-1]=relu(-o), t[0]=1-|o|, t[1]=relu(o), so the
  mask-weighted sample sum collapses to a 5x5 shift window:
     out[pos, (g,c)] = sum_{sy,sx in [-2,2]} A[(g,sy,sx), pos] *
                       xproj_pad2[pos + (sy,sx), (g,c)]
  The 9 (dy,dx) tent-product terms are re-expressed in the 9-product basis
     {m, m*tymn, m*typ, m*txmn, m*txp, m*tymn*txmn, m*tymn*txp,
      m*typ*txmn, m*typ*txp},   tmn=min(o,0), tp=max(o,0),
  whose (constant) scatter matrices fold the basis-change coefficients, so
  the on-chip work is 4 one-scalar tensor_scalar tents (4x DVE mode) and 8
  elementwise products instead of 6 slow-path tent ops and 12 products.
  xproj is padded by 2 (inner ring = conv pad inside the sampling grid,
  outer ring = zeros = grid_sample zero padding), making all window reads
  in-bounds with no boundary special cases.
  final  = out @ w_out + b_out

Layout: channel-major [C on partitions, positions on the free axis], so all
channel contractions are natural matmuls. Matmul operands are bf16 (full PE
rate + FWL weight loads); all accumulation (PSUM) is fp32. x^T arrives via
the DMA xbar transpose (no PE/ACT cost). The output projection is fused
into the apply loop: final = sum_s (A_s (*) img_s) @ w_out accumulates
across the 25 shifts in PSUM; per-shift A-broadcast tiles are consumed
three ways to balance engines: evicted to SBUF by ACT, evicted by the
(otherwise idle) Pool engine, or multiplied straight out of PSUM by DVE.
A-build and apply phases share the schedule (two concurrently-open PSUM
pools) so their dependency bubbles overlap.
"""

from contextlib import ExitStack

import ml_dtypes
import numpy as np

import concourse.bass as bass
import concourse.mybir as mybir
import concourse.tile as tile
from concourse._compat import with_exitstack

N, H, W, C, G, K = 8, 64, 64, 128, 8, 3
GC = C // G            # 16
P = K * K              # 9
POS = H * W            # 4096
HP, WP = H + 2, W + 2            # dwconv pad-1 grid (66)
HP2, WP2 = H + 4, W + 4          # sampling pad-2 grid (68)
EPS = 1e-6
NS = 25                          # 5x5 shift window
NH1, NH2 = 13, 12                # A row split: s in [0,13), [13,25)
R1, R2 = G * NH1, G * NH2        # 104, 96 partition rows of the two A halves
NB = 9                           # tent-product basis size
F32 = mybir.dt.float32
BF16 = mybir.dt.bfloat16
FP8 = mybir.dt.float8e4
NPBF = ml_dtypes.bfloat16
NPF8 = ml_dtypes.float8_e4m3

# packed-constant column layouts (see _host_constants): one DMA per pack
# instead of ~29 serial HWDGE loads, which gated startup by ~12us.
# pkE (bf16): dwdiag | w_in | invc_col | ones_row | b_out_row
_PKE_DW, _PKE_WIN, _PKE_INVC, _PKE_ONES, _PKE_BOUT, _PKE_BINR, _PKE_N = (
    0, P * C, P * C + C, P * C + C + 1, P * C + 2 * C + 1, P * C + 3 * C + 1,
    P * C + 4 * C + 1)
# pkL (bf16): w_offx | w_offy | w_msk | w_out
_PKL_OFFX, _PKL_OFFY, _PKL_MSK, _PKL_WOUT, _PKL_N = (
    0, G * P, 2 * G * P, 3 * G * P, 3 * G * P + C)
# pk8 (fp8e4, exact 0/±small-int constants): m1 | m2 | ea1 | ea2 | egg
_PK8_M1, _PK8_M2, _PK8_EA1, _PK8_EA2, _PK8_EGG, _PK8_N = (
    0, NB * R1, NB * (R1 + R2), NB * (R1 + R2) + NH1 * C,
    NB * (R1 + R2) + (NH1 + NH2) * C, NB * (R1 + R2) + (NH1 + NH2) * C + G * P)
# pkV (fp32 columns): b_in | dw_b | ln_g | ln_b | b_offx | b_offy | b_msk
_PKV_N = 7

CHUNK = 512                      # free-dim chunk for the build phase
NCH = POS // CHUNK               # 8

AF = mybir.ActivationFunctionType
OP = mybir.AluOpType

# Per-shift handling of the A-broadcast PSUM tile in the apply loop (Pool
# cannot touch PSUM, so every evict is on ACT):
#   'A' = ACT evicts to bf16 SBUF, DVE multiplies at 2x
#   'M' = ACT evicts to bf16 SBUF, Pool multiplies (slow but otherwise idle)
#   'D' = DVE multiplies straight out of PSUM (1x, but no evict at all)
APPLY_MODE = ['D', 'A', 'M', 'D', 'A',
              'D', 'A', 'M', 'D', 'A',
              'D', 'M', 'A', 'M', 'D',
              'M', 'D', 'A', 'D', 'A',
              'D', 'M', 'A', 'M', 'D']

# basis-change coefficients: d-term (dy_idx, dx_idx) -> {basis index: coeff}
# with stored tents tmn=min(o,0), tp=max(o,0) and actual taps
# t[-1]=-tmn, t[0]=1+tmn-tp, t[1]=tp.
_COEFF = {
    (0, 0): {5: 1.0},
    (0, 1): {1: -1.0, 5: -1.0, 6: 1.0},
    (0, 2): {6: -1.0},
    (1, 0): {3: -1.0, 5: -1.0, 7: 1.0},
    (1, 1): {0: 1.0, 1: 1.0, 2: -1.0, 3: 1.0, 4: -1.0,
             5: 1.0, 6: -1.0, 7: -1.0, 8: 1.0},
    (1, 2): {4: 1.0, 6: 1.0, 8: -1.0},
    (2, 0): {7: -1.0},
    (2, 1): {2: 1.0, 7: 1.0, 8: -1.0},
    (2, 2): {8: 1.0},
}


# --------------------------------------------------------------------------
# host-side constant matrices
# --------------------------------------------------------------------------

def _host_constants(inputs):
    dw_w = np.asarray(inputs["dw_w"], np.float32)        # (3,3,1,C) [ky,kx]
    w_off = np.asarray(inputs["w_off"], np.float32)      # (C, G*P*2)
    b_off = np.asarray(inputs["b_off"], np.float32)      # (G*P*2,)

    # depthwise weights as 9 diagonal matrices, c-major: [c_row, s, c_col]
    dwdiag = np.zeros((C, P, C), np.float32)
    for s in range(P):
        ky, kx = s // 3, s % 3
        dwdiag[np.arange(C), s, np.arange(C)] = dw_w[ky, kx, 0]

    w_offx = np.ascontiguousarray(w_off[:, 0::2])        # (C, 72)
    w_offy = np.ascontiguousarray(w_off[:, 1::2])
    b_offx = np.ascontiguousarray(b_off[0::2])           # (72,)
    b_offy = np.ascontiguousarray(b_off[1::2])

    # block-ones matrix: one matmul produces the group sums broadcast to
    # all 72 (g,p) rows at once (softmax denominator)
    egg = np.zeros((G * P, G * P), np.float32)
    for g in range(G):
        egg[g * P:(g + 1) * P, g * P:(g + 1) * P] = 1.0

    # A-scatter matrices over the 9-product basis: basis term b of point
    # (g,p) lands in A row (g, s), s = (p%3 + dy)*5 + (p//3 + dx) (x-major
    # p!), weighted by the basis-change coefficient of d-term (dy,dx).
    m1 = np.zeros((G * P, NB, R1), np.float32)
    m2 = np.zeros((G * P, NB, R2), np.float32)
    for (dy, dx), cs in _COEFF.items():
        for g in range(G):
            for p in range(P):
                s = (p % 3 + dy) * 5 + (p // 3 + dx)
                for b, coef in cs.items():
                    if s < NH1:
                        m1[g * P + p, b, g * NH1 + s] += coef
                    else:
                        m2[g * P + p, b, g * NH2 + (s - NH1)] += coef

    # A-broadcast matrices: A row (g, s) -> output row (g*GC + c)
    ea1 = np.zeros((R1, NH1, C), np.float32)
    ea2 = np.zeros((R2, NH2, C), np.float32)
    for g in range(G):
        for sl in range(NH1):
            ea1[g * NH1 + sl, sl, g * GC:(g + 1) * GC] = 1.0
        for sl in range(NH2):
            ea2[g * NH2 + sl, sl, g * GC:(g + 1) * GC] = 1.0

    # ---- pack into 4 DMA-able constants ------------------------------
    pkE = np.zeros((C, _PKE_N), np.float32)
    pkE[:, _PKE_DW:_PKE_DW + P * C] = dwdiag.reshape(C, P * C)
    pkE[:, _PKE_WIN:_PKE_WIN + C] = np.asarray(inputs["w_in"], np.float32)
    pkE[:, _PKE_INVC] = 1.0 / C
    pkE[0, _PKE_ONES:_PKE_ONES + C] = 1.0
    pkE[0, _PKE_BOUT:_PKE_BOUT + C] = np.asarray(inputs["b_out"], np.float32)
    pkE[0, _PKE_BINR:_PKE_BINR + C] = np.asarray(inputs["b_in"], np.float32)

    pkL = np.zeros((C, _PKL_N), np.float32)
    pkL[:, _PKL_OFFX:_PKL_OFFX + G * P] = w_offx
    pkL[:, _PKL_OFFY:_PKL_OFFY + G * P] = w_offy
    pkL[:, _PKL_MSK:_PKL_MSK + G * P] = np.asarray(inputs["w_msk"], np.float32)
    pkL[:, _PKL_WOUT:_PKL_WOUT + C] = np.asarray(inputs["w_out"], np.float32)

    pk8 = np.zeros((C, _PK8_N), np.float32)
    pk8[:G * P, _PK8_M1:_PK8_M1 + NB * R1] = m1.reshape(G * P, NB * R1)
    pk8[:G * P, _PK8_M2:_PK8_M2 + NB * R2] = m2.reshape(G * P, NB * R2)
    pk8[:R1, _PK8_EA1:_PK8_EA1 + NH1 * C] = ea1.reshape(R1, NH1 * C)
    pk8[:R2, _PK8_EA2:_PK8_EA2 + NH2 * C] = ea2.reshape(R2, NH2 * C)
    pk8[:G * P, _PK8_EGG:_PK8_EGG + G * P] = egg

    pkV = np.zeros((C, _PKV_N), np.float32)
    pkV[:, 0] = np.asarray(inputs["b_in"], np.float32)
    pkV[:, 1] = np.asarray(inputs["dw_b"], np.float32)
    pkV[:, 2] = np.asarray(inputs["ln_g"], np.float32)
    pkV[:, 3] = np.asarray(inputs["ln_b"], np.float32)
    pkV[:G * P, 4] = b_off[0::2]
    pkV[:G * P, 5] = b_off[1::2]
    pkV[:G * P, 6] = np.asarray(inputs["b_msk"], np.float32)

    return {"pkE": pkE.astype(NPBF), "pkL": pkL.astype(NPBF),
            "pk8": pk8.astype(NPF8), "pkV": pkV}


# --------------------------------------------------------------------------
# the per-core Tile program
# --------------------------------------------------------------------------

@with_exitstack
def _dcn_tile(ctx: ExitStack, tc: tile.TileContext, io: dict):
    nc = tc.nc
    ctx.enter_context(nc.allow_low_precision(
        reason="bf16 matmul operands; accumulation stays fp32 in PSUM and "
               "in the fp32 sampling accumulator"))

    persist = ctx.enter_context(tc.tile_pool(name="persist", bufs=1))
    temps = ctx.enter_context(tc.tile_pool(name="temps", bufs=3))
    tents = ctx.enter_context(tc.tile_pool(name="tents", bufs=2))

    # ---- packed constant loads ----------------------------------------
    # x^T transpose rides the DMA xbar; two separate landing tiles so each
    # restride copy depends on only its own transpose (transposed writes
    # are tracked whole-tile)
    xt_half = [persist.tile([C, POS // 2], BF16, tag=f"xt_half{h}",
                            name=f"xt_half{h}") for h in range(2)]

    def transpose_half(hf, eng):
        eng.dma_start(
            out=xt_half[hf],
            in_=bass.AP(tensor=io["x"].tensor,
                        offset=io["x"].offset + hf * (POS // 2) * C,
                        ap=[[C, POS // 2], [1, C]]),
            transpose=True)

    pkE_t = persist.tile([C, _PKE_N], BF16, tag="pkE")
    pkL_t = persist.tile([C, _PKL_N], BF16, tag="pkL")
    pk8_t = persist.tile([C, _PK8_N], FP8, tag="pk8")
    pkV_t = persist.tile([C, _PKV_N], F32, tag="pkV")

    # spread the loads over all DMA-issuing queues (ACT/SP HWDGE + Pool
    # SWDGE) so the tile scheduler can't chain them behind each other
    transpose_half(0, nc.sync)
    nc.sync.dma_start(out=pkE_t, in_=io["pkE"])
    transpose_half(1, nc.scalar)
    nc.scalar.dma_start(out=pk8_t, in_=io["pk8"])
    nc.gpsimd.dma_start(out=pkV_t, in_=io["pkV"])
    nc.gpsimd.dma_start(out=pkL_t, in_=io["pkL"])

    # views into the packs (same shapes the rest of the program expects)
    dwdiag = pkE_t[:, _PKE_DW:_PKE_DW + P * C].rearrange(
        "c (s k) -> c s k", k=C)                 # [c_row, s, c_col]
    w_in = pkE_t[:, _PKE_WIN:_PKE_WIN + C]
    invc_col = pkE_t[:, _PKE_INVC:_PKE_INVC + 1]
    ones_row = pkE_t[0:1, _PKE_ONES:_PKE_ONES + C]
    b_out_row = pkE_t[0:1, _PKE_BOUT:_PKE_BOUT + C]
    b_in_row = pkE_t[0:1, _PKE_BINR:_PKE_BINR + C]
    b_in = pkV_t[:, 0:1]
    dw_b = pkV_t[:, 1:2]
    ln_g = pkV_t[:, 2:3]
    ln_b = pkV_t[:, 3:4]
    b_offx = pkV_t[0:G * P, 4:5]
    b_offy = pkV_t[0:G * P, 5:6]
    b_msk = pkV_t[0:G * P, 6:7]
    w_offx = pkL_t[:, _PKL_OFFX:_PKL_OFFX + G * P]
    w_offy = pkL_t[:, _PKL_OFFY:_PKL_OFFY + G * P]
    w_msk = pkL_t[:, _PKL_MSK:_PKL_MSK + G * P]
    w_out = pkL_t[:, _PKL_WOUT:_PKL_WOUT + C]
    m1 = pk8_t[0:G * P, _PK8_M1:_PK8_M1 + NB * R1].rearrange(
        "p (b r) -> p b r", r=R1)
    m2 = pk8_t[0:G * P, _PK8_M2:_PK8_M2 + NB * R2].rearrange(
        "p (b r) -> p b r", r=R2)
    ea1 = pk8_t[0:R1, _PK8_EA1:_PK8_EA1 + NH1 * C].rearrange(
        "r (s c) -> r s c", c=C)
    ea2 = pk8_t[0:R2, _PK8_EA2:_PK8_EA2 + NH2 * C].rearrange(
        "r (s c) -> r s c", c=C)
    egg = pk8_t[0:G * P, _PK8_EGG:_PK8_EGG + G * P]

    eps1 = persist.tile([1, 1], F32)
    nc.vector.memset(eps1, EPS)

    # ---- persistent activations ---------------------------------------
    xt_pad = persist.tile([C, HP, WP], BF16)     # x^T, conv-padded (66x66)
    xpj_pad = persist.tile([C, HP2, WP2], BF16)  # x_proj^T, pad-2 (68x68)
    a1 = [persist.tile([R1, CHUNK], BF16, tag=f"a1_{i}", name=f"a1_{i}")
          for i in range(NCH)]
    a2 = [persist.tile([R2, CHUNK], BF16, tag=f"a2_{i}", name=f"a2_{i}")
          for i in range(NCH)]

    # only the pad rings need zeroing; the interiors are fully overwritten
    nc.vector.memset(xt_pad[:, 0:1, :], 0.0)
    nc.vector.memset(xt_pad[:, HP - 1:HP, :], 0.0)
    nc.vector.memset(xt_pad[:, 1:HP - 1, 0:1], 0.0)
    nc.vector.memset(xt_pad[:, 1:HP - 1, WP - 1:WP], 0.0)
    nc.vector.memset(xpj_pad[:, 0:2, :], 0.0)
    nc.vector.memset(xpj_pad[:, HP2 - 2:HP2, :], 0.0)
    nc.vector.memset(xpj_pad[:, 2:HP2 - 2, 0:2], 0.0)
    nc.vector.memset(xpj_pad[:, 2:HP2 - 2, WP2 - 2:WP2], 0.0)

    # ---- stage 1: restride x^T into the padded grid (DVE 4x copies) ---
    nc.vector.tensor_copy(out=xt_pad[:, 1:1 + H // 2, 1:1 + W],
                          in_=xt_half[0])
    nc.vector.tensor_copy(out=xt_pad[:, 1 + H // 2:1 + H, 1:1 + W],
                          in_=xt_half[1])

    # ---- stage 2: x_proj into the pad-2 grid --------------------------
    with tc.tile_pool(name="ps_s12", bufs=2, space="PSUM") as psum:
        for ch in range(NCH):        # chunk = 8 h-rows
            h0 = ch * 8
            rhs = xt_pad[:, 1 + h0:1 + h0 + 8, 1:1 + W]
            ps = psum.tile([C, CHUNK], F32, tag="ps_proj")
            nc.tensor.matmul(ps, w_in, rhs, start=True, stop=True)
            nc.scalar.activation(
                out=xpj_pad[:, 2 + h0:2 + h0 + 8, 2:2 + W],
                in_=ps.rearrange("c (a b) -> c a b", b=W),
                func=AF.Identity, bias=b_in, scale=1.0)

    # ---- stages 3+4, interleaved ---------------------------------------
    # Two PSUM pools, both open across the whole fused phase so the
    # scheduler can interleave A-building and applying freely.
    with tc.tile_pool(name="ps3", bufs=1, space="PSUM") as psum, \
            tc.tile_pool(name="ps4", bufs=1, space="PSUM") as psum4:

        def build_a(ch):
            """dwconv + the [1,512] LayerNorm stats sub-chain.  Emitted
            BEFORE apply(ch-1) so the long serial LN chain percolates
            through ACT/DVE/Pool while PE grinds the apply matmuls."""
            h0 = ch * 8
            # depthwise conv via 9 diagonal matmuls
            psA = psum.tile([C, CHUNK], F32, tag="psdw", bufs=1, name="psA")
            for s in range(P):
                ky, kx = s // 3, s % 3
                rhs = xt_pad[:, h0 + ky:h0 + ky + 8, kx:kx + W]
                nc.tensor.matmul(psA, dwdiag[:, s, :], rhs,
                                 start=(s == 0), stop=(s == P - 1))
            x1c = temps.tile([C, CHUNK], BF16, tag="x1c")
            nc.scalar.activation(out=x1c, in_=psA, func=AF.Identity,
                                 bias=dw_b, scale=1.0)

            # LayerNorm stats over channels (partition dim) via 1/C cols;
            # x1sq on the Pool engine so it doesn't queue behind the
            # previous chunk's apply muls on DVE.
            x1sq = temps.tile([C, CHUNK], BF16, tag="scr")
            nc.gpsimd.tensor_mul(out=x1sq, in0=x1c, in1=x1c)
            pstat = psum.tile([33, CHUNK], F32, tag="psdw", bufs=1,
                              name="pstat")
            nc.tensor.matmul(pstat[0:1, :], invc_col, x1c, start=True,
                             stop=True)
            mean_r = temps.tile([1, CHUNK], BF16, tag="mean_r")
            nc.scalar.copy(out=mean_r, in_=pstat[0:1, :])
            nc.tensor.matmul(pstat[32:33, :], invc_col, x1sq, start=True,
                             stop=True)
            tmp_r = temps.tile([1, CHUNK], F32, tag="tmp_r")
            nc.scalar.activation(out=tmp_r, in_=pstat[0:1, :],
                                 func=AF.Square)
            nc.vector.tensor_sub(out=tmp_r, in0=pstat[32:33, :], in1=tmp_r)
            nc.scalar.activation(out=tmp_r, in_=tmp_r, func=AF.Sqrt,
                                 bias=eps1, scale=1.0)            # std
            rstd_r = temps.tile([1, CHUNK], BF16, tag="rstd_r")
            nc.vector.reciprocal(out=rstd_r, in_=tmp_r)           # rstd
            mrs_r = temps.tile([1, CHUNK], BF16, tag="mrs_r")
            nc.vector.tensor_mul(out=mrs_r, in0=mean_r, in1=rstd_r)
            return x1c, rstd_r, mrs_r

        def build_b(ch, x1c, rstd_r, mrs_r):
            h0 = ch * 8
            psR = psum.tile([C, CHUNK], F32, tag="psoff", bufs=2, name="psR")
            nc.tensor.matmul(psR, ones_row, rstd_r, start=True, stop=True)
            zc = temps.tile([C, CHUNK], F32, tag="scr2")
            nc.vector.tensor_mul(out=zc, in0=x1c, in1=psR)
            psM = psum.tile([C, CHUNK], F32, tag="psoff", bufs=2, name="psM")
            nc.tensor.matmul(psM, ones_row, mrs_r, start=True, stop=True)
            nc.vector.tensor_sub(out=zc, in0=zc, in1=psM)
            x1gc = temps.tile([C, CHUNK], BF16, tag="x1gc")
            nc.scalar.activation(out=x1gc, in_=zc, func=AF.Gelu,
                                 bias=ln_b, scale=ln_g)

            # offset projections; tents as one-scalar tensor_scalar ops on
            # bf16 SBUF evictions (4x DVE mode)
            psX = psum.tile([G * P, CHUNK], F32, tag="psoff", bufs=2,
                            name="psX")
            nc.tensor.matmul(psX, w_offx, x1gc, start=True, stop=True)
            oxs = tents.tile([G * P, CHUNK], BF16, tag="oxs")
            nc.scalar.activation(out=oxs, in_=psX, func=AF.Identity,
                                 bias=b_offx, scale=1.0)
            txmn = tents.tile([G * P, CHUNK], BF16, tag="txmn")
            txp = tents.tile([G * P, CHUNK], BF16, tag="txp")
            nc.vector.tensor_scalar(out=txmn, in0=oxs, scalar1=0.0,
                                    scalar2=None, op0=OP.min)
            nc.vector.tensor_scalar(out=txp, in0=oxs, scalar1=0.0,
                                    scalar2=None, op0=OP.max)
            psY = psum.tile([G * P, CHUNK], F32, tag="psoff", bufs=2,
                            name="psY")
            nc.tensor.matmul(psY, w_offy, x1gc, start=True, stop=True)
            oys = tents.tile([G * P, CHUNK], BF16, tag="oys")
            nc.scalar.activation(out=oys, in_=psY, func=AF.Identity,
                                 bias=b_offy, scale=1.0)
            tymn = tents.tile([G * P, CHUNK], BF16, tag="tymn")
            typ = tents.tile([G * P, CHUNK], BF16, tag="typ")
            nc.vector.tensor_scalar(out=tymn, in0=oys, scalar1=0.0,
                                    scalar2=None, op0=OP.min)
            nc.vector.tensor_scalar(out=typ, in0=oys, scalar1=0.0,
                                    scalar2=None, op0=OP.max)

            # normalized mask: e_n = exp(l + b) / group sum
            psE = psum.tile([G * P, CHUNK], F32, tag="psoff", bufs=2,
                            name="psE")
            nc.tensor.matmul(psE, w_msk, x1gc, start=True, stop=True)
            ec = temps.tile([G * P, CHUNK], BF16, tag="ec")
            nc.scalar.activation(out=ec, in_=psE, func=AF.Exp,
                                 bias=b_msk, scale=1.0)
            psB = psum.tile([G * P, CHUNK], F32, tag="psoff", bufs=2,
                            name="psB")
            nc.tensor.matmul(psB, egg, ec, start=True, stop=True)
            rec_b = temps.tile([G * P, CHUNK], BF16, tag="rec_b")
            nc.vector.reciprocal(out=rec_b, in_=psB)
            nc.vector.tensor_mul(out=ec, in0=ec, in1=rec_b)

            # 8 basis products (all-bf16 SBUF tensor_tensor, 2x mode)
            b1 = tents.tile([G * P, CHUNK], BF16, tag="b1")
            b2 = tents.tile([G * P, CHUNK], BF16, tag="b2")
            b3 = tents.tile([G * P, CHUNK], BF16, tag="b3")
            b4 = tents.tile([G * P, CHUNK], BF16, tag="b4")
            b5 = tents.tile([G * P, CHUNK], BF16, tag="b5")
            b6 = tents.tile([G * P, CHUNK], BF16, tag="b6")
            b7 = tents.tile([G * P, CHUNK], BF16, tag="b7")
            b8 = tents.tile([G * P, CHUNK], BF16, tag="b8")
            nc.vector.tensor_mul(out=b1, in0=ec, in1=tymn)
            nc.vector.tensor_mul(out=b2, in0=ec, in1=typ)
            nc.vector.tensor_mul(out=b3, in0=ec, in1=txmn)
            nc.vector.tensor_mul(out=b4, in0=ec, in1=txp)
            nc.gpsimd.tensor_mul(out=b5, in0=b1, in1=txmn)
            nc.vector.tensor_mul(out=b6, in0=b1, in1=txp)
            nc.gpsimd.tensor_mul(out=b7, in0=b2, in1=txmn)
            nc.vector.tensor_mul(out=b8, in0=b2, in1=txp)
            basis = [ec, b1, b2, b3, b4, b5, b6, b7, b8]

            psA1 = psum.tile([R1, CHUNK], F32, tag="psa", bufs=1,
                             name="psA1")
            for b in range(NB):
                nc.tensor.matmul(psA1, m1[:, b, :], basis[b],
                                 start=(b == 0), stop=(b == NB - 1))
            nc.scalar.copy(out=a1[ch], in_=psA1)
            psA2 = psum.tile([R2, CHUNK], F32, tag="psa", bufs=1,
                             name="psA2")
            for b in range(NB):
                nc.tensor.matmul(psA2, m2[:, b, :], basis[b],
                                 start=(b == 0), stop=(b == NB - 1))
            nc.scalar.copy(out=a2[ch], in_=psA2)

        def apply_chunk(ch):
            h0 = ch * 8            # 8 h-rows per 512-pos chunk
            # pos-major output accumulator: outp[p, q, c] is position
            # ch*512 + q*128 + p.  Initialized with the bias via rank-1
            # matmuls; each shift's product tile t becomes the lhsT of four
            # 128-col projection matmuls, so no output transpose is needed.
            outp = psum4.tile([C, 4, C], F32, tag="psoacc", bufs=1,
                              name="outp")
            for q in range(4):
                nc.tensor.matmul(outp[:, q, :], ones_row, b_out_row,
                                 start=True, stop=False)
            def product(s, mode):
                sy, sx = s // 5 - 2, s % 5 - 2
                if s < NH1:
                    lhsT, arows = ea1[:, s, :], a1[ch]
                else:
                    lhsT, arows = ea2[:, s - NH1, :], a2[ch]
                psBc = psum4.tile([C, CHUNK], F32, tag="psab", bufs=3,
                                  name="psBc")
                nc.tensor.matmul(psBc, lhsT, arows, start=True, stop=True)
                row = 2 + sy + h0
                img = xpj_pad[:, row:row + 8, 2 + sx:2 + sx + W]
                if mode == 'D':      # multiply straight out of PSUM
                    t = temps.tile([C, CHUNK], BF16, tag="t_app")
                    nc.vector.tensor_mul(out=t, in0=psBc, in1=img)
                elif mode == 'A':
                    ab = temps.tile([C, CHUNK], BF16, tag="ab_a")
                    nc.scalar.copy(out=ab, in_=psBc)
                    t = temps.tile([C, CHUNK], BF16, tag="t_app")
                    nc.vector.tensor_mul(out=t, in0=ab, in1=img)
                else:                # Pool multiplies; wout deferred
                    ab = temps.tile([C, CHUNK], BF16, tag="ab_p", bufs=6)
                    nc.scalar.copy(out=ab, in_=psBc)
                    t = temps.tile([C, CHUNK], BF16, tag="t_appm", bufs=6)
                    nc.gpsimd.tensor_mul(out=t, in0=ab, in1=img)
                return t

            def wout(t, last):
                for q in range(4):
                    nc.tensor.matmul(outp[:, q, :], t[:, q * C:(q + 1) * C],
                                     w_out, start=False, stop=last)

            # The slow Pool products would stall the in-order PSUM
            # accumulation, so their ea/evict/mul run interleaved with the
            # main pass but their wout groups accumulate at the very end.
            # final chunk: no later build overlaps the slow Pool muls, so
            # run its evicted shifts on ACT+DVE instead
            mode_l = APPLY_MODE if ch < NCH - 1 else [
                'A' if m == 'M' else m for m in APPLY_MODE]
            m_shifts = [s for s in range(NS) if mode_l[s] == 'M']
            o_shifts = [s for s in range(NS) if mode_l[s] != 'M']
            mq = list(m_shifts)
            m_prods = []
            for i, s in enumerate(o_shifts):
                if i % 3 == 0 and mq:
                    sm = mq.pop(0)
                    m_prods.append(product(sm, 'M'))
                wout(product(s, mode_l[s]),
                     last=(not m_prods and not mq and s == o_shifts[-1]))
            while mq:
                m_prods.append(product(mq.pop(0), 'M'))
            for j, t in enumerate(m_prods):
                wout(t, last=(j == len(m_prods) - 1))

            osb = temps.tile([C, 4, C], F32, tag="osb")
            if ch % 2 == 0:
                nc.vector.tensor_copy(out=osb, in_=outp)
            else:
                nc.scalar.copy(out=osb, in_=outp)
            pos0 = ch * CHUNK
            nc.sync.dma_start(
                out=bass.AP(tensor=io["out"].tensor,
                            offset=io["out"].offset + pos0 * C,
                            ap=[[C, C], [C * C, 4], [1, C]]),
                in_=osb)

        fronts = {0: build_a(0)}
        for ch in range(NCH):
            if ch + 1 < NCH:
                fronts[ch + 1] = build_a(ch + 1)
            if ch > 0:
                apply_chunk(ch - 1)
            build_b(ch, *fronts.pop(ch))
        apply_chunk(NCH - 1)


# --------------------------------------------------------------------------
# bass module build + public entry point
# --------------------------------------------------------------------------

# Hardware TPB instructions carry exactly ONE sync-wait slot (the
# NEURON_ISA_TPB_EVENTS struct).  Tile can emit several waits on one BIR
# instruction; walrus splits matmult waits across the LDWEIGHTS/MATMULT
# pair, but single-struct ops (Activation, ...) fail codegen with "Too many
# sync wait commands".  Move surplus waits onto standalone same-engine
# EventSemaphore instructions inserted immediately before the offender.
def _wait_cap(ins):
    t = type(ins).__name__
    if t == "InstEventSemaphore":
        return None
    return 1


def _split_surplus_waits(nc):
    import bass_rust
    n = 0
    for bb in nc.m.functions[0].blocks:
        out = []
        for ins in bb.instructions:
            si = getattr(ins, "sync_info", None)
            cap = _wait_cap(ins)
            if si is not None and cap is not None and len(si.on_wait) > cap:
                waits = list(si.on_wait)
                for i, w in enumerate(waits[:-cap]):
                    ev = mybir.InstEventSemaphore(
                        name=f"{ins.name}_xw{i}", ins=[], outs=[])
                    ev.engine = ins.engine
                    ev.sync_info = bass_rust.SyncInfo(on_wait=[w],
                                                     on_update=[])
                    nc.register_instruction(ev)
                    out.append(ev)
                    n += 1
                ins.sync_info = bass_rust.SyncInfo(
                    on_wait=waits[-cap:], on_update=list(si.on_update))
            out.append(ins)
        bb.instructions = out
    return n


_CACHED = {}


def _build_bass():
    if "nc" in _CACHED:
        return _CACHED["nc"]
    nc = bass.Bass()
    io = {}
    specs = {
        "x": ((POS, C), BF16),
        "pkE": ((C, _PKE_N), BF16),
        "pkL": ((C, _PKL_N), BF16),
        "pk8": ((C, _PK8_N), FP8),
        "pkV": ((C, _PKV_N), F32),
    }
    for name, (shape, dt) in specs.items():
        io[name] = nc.dram_tensor(name, list(shape), dt,
                                  kind="ExternalInput").ap()
    io["out"] = nc.dram_tensor("out", [POS, C], F32,
                               kind="ExternalOutput").ap()
    with tile.TileContext(nc) as tc:
        _dcn_tile(tc, io)
    _split_surplus_waits(nc)
    _CACHED["nc"] = nc
    return nc


def make_in_maps(inputs):
    consts = _host_constants(inputs)
    x = np.asarray(inputs["x"], np.float32).reshape(N, POS, C).astype(NPBF)
    return [{**consts, "x": np.ascontiguousarray(x[i])} for i in range(N)]


def kernel(**inputs):
    nc = _build_bass()
    in_maps = make_in_maps(inputs)
    from concourse.bass_utils import run_bass_kernel_spmd
    res = run_bass_kernel_spmd(nc, in_maps, list(range(N)))
    out = np.stack([res.results[i]["out"] for i in range(N)])
    return out.reshape(N, H, W, C).astype(np.float32)

